# revision 41
# baseline (speedup 1.0000x reference)
"""Trainium2 Bass kernel for a full attention block (B=4, S=2048, H=1024, 16 heads).

Sharding: 8 cores = (batch b = core//2) x (query-half qh = core%2).
Each core computes the complete pipeline for its 1024 query rows of batch b:
QKV projections, 16-head attention over all 2048 keys, output projection,
residual add and layernorm.  No cross-core communication; the host slices
inputs and concatenates the 8 disjoint output shards.

v3 (stall-shaped PE stream), tuned from v2's trace (407us) where the PE
pipe was 100% occupied but ~20% of it was idle gaps + cold-clock matmuls
caused by in-order head-of-line stalls:
  - All four projections and the probs@V / ctx@Wo matmuls run as fp8e4
    DoubleRow matmuls (two 128-deep k-tiles per instruction).  Weights are
    pre-scaled x4 so their sigma ~ 0.124 sits in fp8e4's normal range; the
    compensation folds into the softmax exp scale and the residual add.
    Scores (contraction = head dim 64, un-pairable) stay bf16 and row-tile
    the two heads of a pair into array rows 0-63 / 64-127 (concurrent).
  - Softmax probs are fp8e5; exps split across ACT (Exp activation) and
    DVE (Schraudolph bitcast exp) by DVE_KTS; V PSUM drains ride the ACT
    engine (Copy lives in every act table set) to balance the two.
  - The PE stream is shaped around its in-order execution: per pair, ctx
    matmuls trail the score stream by LEAD k-tiles (late enough that the
    previous chunk's PSUM drain finished - no head-of-line stall - early
    enough to fill the exp-rate-limited score gaps), and Q/K/V projection
    tiles for FUTURE pairs are sprinkled between score k-tiles as filler.
  - ctx accumulates unnormalized with a ones column at V slot 0, so the
    softmax denominator L lands in PSUM partition 0 where the Pool
    partition_broadcast can read the DVE reciprocal directly (no
    cross-partition DMA bounce on the critical path); the 1/L multiply is
    fused into the PSUM->SBUF fp8 drain whose CTX DMA is off-path.
  - The epilogue (out-proj + residual + LN) is split: qt 0-3's matmul +
    residual interleave with pair 7's c=1 attention; layernorm (whose Sqrt
    would thrash the ACT exp table) runs after the last exp.
  - Weights arrive host-pre-swizzled so each SBUF weight load is one
    contiguous DMA descriptor per partition.
"""

import numpy as np
import ml_dtypes

B, S, H, NH, DH = 4, 2048, 1024, 16, 64
P = 128
NCORES = 8
SQ = 1024        # query rows per core
EPS = 1e-12

W_SCALE = 4.0                       # weight pre-scale into fp8e4 range
EXP_SCALE = 0.125 / (W_SCALE * W_SCALE)   # 1/sqrt(DH) / (Wq*Wk scales)
OUT_SCALE = 1.0 / (W_SCALE * W_SCALE)     # undo Wv*Wo scales after out-proj
LOG2E = 1.4426950408889634
# Schraudolph exp -> e5m2: i8 = sp * EXP_A + EXP_B, bitcast to fp8e5.
# value = 2^((i-60)/4); a = 4*EXP_SCALE*log2(e); b centers the sawtooth
# (-0.17) and compensates truncation (+0.5 if the convert truncates).
EXP_A = 4.0 * EXP_SCALE * LOG2E
EXP_B = 60.0 - 0.172
DVE_KTS = (2, 5, 8, 11, 14)         # score k-tiles whose exp runs on DVE

_CACHE = {}


def _build_program(use_bias, use_affine):
    from concourse import bacc, tile, mybir

    f32 = mybir.dt.float32
    bf16 = mybir.dt.bfloat16
    fp8 = mybir.dt.float8e4
    fp8e5 = mybir.dt.float8e5
    i8 = mybir.dt.int8
    AF = mybir.ActivationFunctionType
    OP = mybir.AluOpType
    DR = mybir.MatmulPerfMode.DoubleRow

    HP = H + P if use_bias else H   # padded contraction (bias ones row)
    KO = HP // P                    # projection contraction subtiles
    KC = H // P + (1 if use_bias else 0)  # out-proj contraction subtiles

    nc = bacc.Bacc("TRN2", target_bir_lowering=False, debug=False,
                   num_devices=NCORES)

    # weights arrive pre-swizzled from the host so every SBUF weight load is
    # one contiguous descriptor per partition (instead of a KO-fragment
    # descriptor storm that serializes the DMA queues at startup)
    xT_d = nc.dram_tensor("xT", [HP, SQ], fp8, kind="ExternalInput").ap()
    xq_d = nc.dram_tensor("xq", [SQ, H], f32, kind="ExternalInput").ap()
    wqT_d = nc.dram_tensor("wqT", [P, 8, KO, P], fp8,
                           kind="ExternalInput").ap()
    wkT_d = nc.dram_tensor("wkT", [P, 8, KO, P], fp8,
                           kind="ExternalInput").ap()
    wvT_d = nc.dram_tensor("wvT", [P, 2, KO, 512], fp8,
                           kind="ExternalInput").ap()
    woT_d = nc.dram_tensor("woT", [P, KC, H], fp8, kind="ExternalInput").ap()
    if use_affine:
        gam_d = nc.dram_tensor("gam", [P, H], f32, kind="ExternalInput").ap()
        bet_d = nc.dram_tensor("bet", [P, H], f32, kind="ExternalInput").ap()
    out_d = nc.dram_tensor("out", [SQ, H], f32, kind="ExternalOutput").ap()

    xT_r = xT_d.rearrange("(o p) s -> p o s", p=P)      # [128, KO, 1024]

    # K/V are computed for only the core's OWN 1024 keys (its xT columns);
    # the pair-peer's half arrives via a pair-wise DRAM AllGather.  Both
    # cores stage "their first 1024 columns", so the gathered order
    # [even-core keys | odd-core keys] is canonical and identical on both
    # cores with no rank-dependent code; attention is key-order invariant,
    # and KT/V use the same canonical order so probs x V pairing stays
    # consistent.
    CC_GROUPS = [[2 * b, 2 * b + 1] for b in range(NCORES // 2)]
    xq_r = xq_d.rearrange("(t p) j -> p t j", p=P)      # [128, 8, 1024]
    out_r = out_d.rearrange("(t p) j -> p t j", p=P)

    with tile.TileContext(nc) as tc:
        with tc.tile_pool(name="pers", bufs=1) as pers, \
             tc.tile_pool(name="probs", bufs=20) as probs_pool, \
             tc.tile_pool(name="att1", bufs=2) as att1, \
             tc.tile_pool(name="wop", bufs=1) as wop:
            KT = pers.tile([P, 8, S], bf16)       # [p, jt, s]; j = jt*128+p
            QT = pers.tile([P, 8, SQ], bf16)
            V = pers.tile([P, 16, NH, 66], fp8)   # [k%128, k//128, head, 1@0|d]
            CTX = pers.tile([P, KC, SQ], fp8)     # ctxT (+ ones row subtile)

            # ones column at V slot 0 -> softmax denominator L lands in PSUM
            # partition 0, where partition_broadcast can read it directly
            # (no cross-partition DMA bounce on the critical path)
            nc.gpsimd.memset(V[:, :, :, 0:1], 1.0)
            if use_bias:
                nc.gpsimd.memset(CTX[:, KC - 1, :], 0.0)
                # ones value W_SCALE so bias row (x4) reaches 16*bo
                nc.gpsimd.memset(CTX[0:1, KC - 1, :], W_SCALE)

            with tc.tile_pool(name="spsum", bufs=2, space="PSUM") as spsum, \
                 tc.tile_pool(name="ppsum", bufs=2, space="PSUM") as ppsum, \
                 tc.tile_pool(name="cpsum", bufs=2, space="PSUM") as cpsum:

                def emit_pair(pair, fillers=(), crange=(0, 1)):
                    hA, hB = 2 * pair, 2 * pair + 1
                    jt = pair
                    fillers = list(fillers)
                    nfill = len(fillers)
                    nslots = 16 * len(crange)
                    # two q-chunk halves; each [128,1024] score psum tile
                    # holds both heads (cols 0:512 head A, 512: head B) so a
                    # single exp op serves the pair; probs tiles pack kt
                    # PAIRS ([p, 2, 1024]) for the DoubleRow ctx matmuls.
                    #
                    # All score matmuls for both chunks are emitted BEFORE
                    # any ctx matmul: the PE executes its stream in order,
                    # so a ctx matmul waiting on a PSUM bank (previous
                    # chunk's drain) must not sit in front of independent
                    # score work (head-of-line blocking).  `fillers` are
                    # projection-tile thunks for upcoming pairs, sprinkled
                    # between score k-tiles: the score stream is rate-limited
                    # by the exp engines via the two sp PSUM banks, and the
                    # filler matmuls keep the PE busy (and its pstate hot)
                    # through each exp wait.
                    prs = {}
                    slot = 0
                    done = 0
                    LEAD = 6   # ctx matmuls trail the score stream by 6
                    #            k-tiles: late enough that the previous
                    #            chunk's PSUM drain is finished (no
                    #            head-of-line stall), early enough to fill
                    #            the exp-rate-limited score gaps

                    def ctx_mm(c, m):
                        t2, hi = m // 2, m % 2
                        h = (hA, hB)[hi]
                        base = hi * 64
                        nc.tensor.matmul(
                            ctxpss[c][h][:],
                            V[:, 2 * t2:2 * t2 + 2, h, 0:65],
                            prs[c, t2][:, 0:2, base * 8:base * 8 + 512],
                            start=(t2 == 0), stop=(t2 == 7),
                            perf_mode=DR)

                    ctxpss = {}
                    for c in crange:
                        cs = slice(c * 512, (c + 1) * 512)
                        ctxpss[c] = {h: cpsum.tile([65, 512], f32, tag="ctx",
                                                   name=f"ctxps_{h}_{c}")
                                     for h in (hA, hB)}
                        for kt in range(16):
                            t2, j2 = kt // 2, kt % 2
                            if j2 == 0:
                                prs[c, t2] = probs_pool.tile(
                                    [P, 2, SQ], fp8e5, tag="pt",
                                    name=f"pr_{t2}_{c}")
                            pr = prs[c, t2]
                            sp = spsum.tile([P, SQ], f32, tag="sp",
                                            name=f"sps_{kt}_{c}")
                            for h in (hA, hB):
                                base = (h % 2) * 64
                                nc.tensor.matmul(
                                    sp[:, base * 8:base * 8 + 512],
                                    KT[base:base + 64, jt, kt * P:(kt + 1) * P],
                                    QT[base:base + 64, jt, cs],
                                    start=True, stop=True)
                            if kt in DVE_KTS:
                                nc.vector.tensor_scalar(
                                    pr[:, j2, :].bitcast(i8), sp[:],
                                    float(EXP_A), float(EXP_B),
                                    OP.mult, OP.add)
                            else:
                                nc.scalar.activation(pr[:, j2, :], sp[:],
                                                     AF.Exp,
                                                     scale=float(EXP_SCALE))
                            if kt >= LEAD:
                                ctx_mm(c, kt - LEAD)
                            slot += 1
                            want = (nfill * slot) // nslots
                            while done < want:
                                fillers[done]()
                                done += 1
                        for m in range(16 - LEAD, 16):
                            ctx_mm(c, m)
                    while done < nfill:
                        fillers[done]()
                        done += 1
                    for c in crange:
                        cs = slice(c * 512, (c + 1) * 512)
                        ctxps = ctxpss[c]
                        # denominator L = psum row 0 (V ones column at slot
                        # 0): copy to SBUF partition 0 (the custom recip op
                        # misreads PSUM on hw), 1/L, Pool-broadcast straight
                        # from partition 0, and fold the multiply into the
                        # PSUM->SBUF fp8 drain; both ctx halves bounce
                        # through an SBUF stage whose DMA to CTX is off the
                        # critical path.
                        for h in (hA, hB):
                            base = (h % 2) * 64
                            lstage = att1.tile([1, 512], f32, tag="lstage",
                                               name=f"lstage_{h}_{c}")
                            nc.vector.tensor_copy(lstage[:],
                                                  ctxps[h][0:1, :])
                            lrec = att1.tile([1, 512], f32, tag="lrec",
                                             name=f"lrec_{h}_{c}")
                            nc.vector.reciprocal_approx_fast(lrec[:],
                                                             lstage[:])
                            lrep = att1.tile([80, 512], f32, tag="lrep",
                                             name=f"lrep_{h}_{c}")
                            nc.gpsimd.partition_broadcast(lrep[:],
                                                          lrec[0:1, :])
                            # stt over rows 0:65 (row 0 computes L*(1/L),
                            # discarded) -- PSUM reads must start at an
                            # aligned partition, so cover the full range and
                            # DMA only rows 1:65 into CTX
                            cstage = att1.tile([65, 512], fp8,
                                               tag="cstage",
                                               name=f"cstage_{h}_{c}")
                            nc.vector.scalar_tensor_tensor(
                                cstage[0:65, :], ctxps[h][0:65, :],
                                1.0, lrep[0:65, :], OP.bypass, OP.mult)
                            nc.sync.dma_start(CTX[base:base + 64, jt, cs],
                                              cstage[1:65, :])

                with tc.tile_pool(name="proj", bufs=1) as projp, \
                     tc.tile_pool(name="wstr", bufs=2) as wstr, \
                     tc.tile_pool(name="wvstr", bufs=1) as wvstr, \
                     tc.tile_pool(name="ccd", bufs=2, space="DRAM") as ccd:
                    XT = projp.tile([P, KO, SQ], fp8)
                    # issue the first Q-weight tile ahead of the bulk XT
                    # chunks so the PE can start at chunk 1, not chunk 8
                    wq0_t = wstr.tile([P, KO, P], fp8, tag="w", name="wq_0")
                    nc.sync.dma_start(wq0_t[:], wqT_d[:, 0, :, :])
                    # per-subtile loads so the first projection matmul can
                    # start as soon as chunk 0 lands; chunk 0 is split into
                    # 4 partition-range DMAs so it arrives ~4x sooner
                    for ps in range(0, P, 32):
                        nc.sync.dma_start(XT[ps:ps + 32, 0, :],
                                          xT_r[ps:ps + 32, 0, :])
                    for ko in range(1, KO):
                        nc.sync.dma_start(XT[:, ko, :], xT_r[:, ko, :])

                    def proj_mm(ps, w_t, cs):
                        for k2 in range(KO // 2):
                            nc.tensor.matmul(
                                ps[:], w_t[:, 2 * k2:2 * k2 + 2, :],
                                XT[:, 2 * k2:2 * k2 + 2, cs],
                                start=(k2 == 0),
                                stop=(KO % 2 == 0 and k2 == KO // 2 - 1),
                                perf_mode=DR)
                        if KO % 2:
                            nc.tensor.matmul(
                                ps[:], w_t[:, KO - 1, :], XT[:, KO - 1, cs],
                                start=False, stop=True)

                    def qk_thunks(jt, wq_t=None):
                        # weight DMAs issue now (well ahead); the per-tile
                        # matmul+drain emissions are returned as thunks for
                        # interleaving into the previous pair's score loop
                        if wq_t is None:
                            wq_t = wstr.tile([P, KO, P], fp8, tag="w",
                                             name=f"wq_{jt}")
                            nc.sync.dma_start(wq_t[:],
                                              wqT_d[:, jt, :, :])
                        wk_t = wstr.tile([P, KO, P], fp8, tag="w",
                                         name=f"wk_{jt}")
                        nc.sync.dma_start(wk_t[:],
                                          wkT_d[:, jt, :, :])

                        def q_tile(sc):
                            ps = ppsum.tile([P, 512], f32, tag="pp",
                                            name=f"qps_{jt}_{sc}")
                            proj_mm(ps, wq_t, slice(sc * 512, sc * 512 + 512))
                            nc.vector.tensor_copy(
                                QT[:, jt, sc * 512:(sc + 1) * 512], ps[:])

                        def k_tile(sc):
                            ps = ppsum.tile([P, 512], f32, tag="pp",
                                            name=f"kps_{jt}_{sc}")
                            proj_mm(ps, wk_t, slice(sc * 512, sc * 512 + 512))
                            nc.vector.tensor_copy(
                                KT[:, jt, sc * 512:(sc + 1) * 512], ps[:])

                        def k_gather():
                            # exchange K halves with the pair peer; the
                            # canonical [even|odd] readback overwrites both
                            # halves, so every consumer waits on it
                            ks = ccd.tile([P, SQ], bf16, tag="kst",
                                          name=f"kst_{jt}")
                            kg = ccd.tile([2, P, SQ], bf16, tag="kg",
                                          name=f"kg_{jt}")
                            nc.sync.dma_start(ks[:], KT[:, jt, 0:SQ])
                            nc.gpsimd.collective_compute(
                                "AllGather", OP.bypass,
                                replica_groups=CC_GROUPS,
                                ins=[ks[:]], outs=[kg[:]])
                            nc.sync.dma_start(KT[:, jt, 0:SQ], kg[0, :, :])
                            nc.sync.dma_start(KT[:, jt, SQ:S], kg[1, :, :])

                        return ([lambda sc=sc: q_tile(sc) for sc in range(2)]
                                + [lambda sc=sc: k_tile(sc)
                                   for sc in range(2)]
                                + [k_gather])

                    def v_thunks(jc):
                        wv_t = wvstr.tile([P, KO, 512], fp8, tag="wv",
                                          name=f"wv_{jc}")
                        nc.sync.dma_start(
                            wv_t[:], wvT_d[:, jc, :, :])

                        def v_tile(st):
                            ps = ppsum.tile([P, 512], f32, tag="pp",
                                            name=f"vps_{st}_{jc}")
                            for k2 in range(KO // 2):
                                nc.tensor.matmul(
                                    ps[:],
                                    XT[:, 2 * k2:2 * k2 + 2, st * P:(st + 1) * P],
                                    wv_t[:, 2 * k2:2 * k2 + 2, :],
                                    start=(k2 == 0),
                                    stop=(KO % 2 == 0 and k2 == KO // 2 - 1),
                                    perf_mode=DR)
                            if KO % 2:
                                nc.tensor.matmul(
                                    ps[:], XT[:, KO - 1, st * P:(st + 1) * P],
                                    wv_t[:, KO - 1, :],
                                    start=False, stop=True)
                            # V drain on ACT (Copy is resident in every act
                            # table set) -- DVE is the more loaded engine
                            nc.scalar.activation(
                                V[:, st, jc * 8:(jc + 1) * 8, 1:65],
                                ps[:].rearrange("p (h d) -> p h d", d=64),
                                AF.Copy)

                        def v_gather():
                            vs = ccd.tile([P, 8, 8, 66], fp8, tag="vst",
                                          name=f"vst_{jc}")
                            vg = ccd.tile([2, P, 8, 8, 66], fp8, tag="vg",
                                          name=f"vg_{jc}")
                            nc.sync.dma_start(
                                vs[:], V[:, 0:8, jc * 8:(jc + 1) * 8, :])
                            nc.gpsimd.collective_compute(
                                "AllGather", OP.bypass,
                                replica_groups=CC_GROUPS,
                                ins=[vs[:]], outs=[vg[:]])
                            nc.sync.dma_start(
                                V[:, 0:8, jc * 8:(jc + 1) * 8, :],
                                vg[0, :, :, :, :])
                            nc.sync.dma_start(
                                V[:, 8:16, jc * 8:(jc + 1) * 8, :],
                                vg[1, :, :, :, :])

                        return ([lambda st=st: v_tile(st) for st in range(8)]
                                + [v_gather])

                    # pair 0 needs QK(0) + V half 0 immediately; everything
                    # later streams in as filler.  V's second half feeds only
                    # pairs 4-7: its tiles fill pairs 2-3.
                    for th in qk_thunks(0, wq_t=wq0_t):
                        th()
                    for th in v_thunks(0):
                        th()
                    emit_pair(0, qk_thunks(1))
                    emit_pair(1, qk_thunks(2))
                    v1 = v_thunks(1)
                    emit_pair(2, qk_thunks(3) + v1[:6])
                    emit_pair(3, v1[6:] + qk_thunks(4))
                    emit_pair(4, qk_thunks(5))
                    emit_pair(5, qk_thunks(6))
                    emit_pair(6, qk_thunks(7))

                # XT freed; stream the output-projection weight during the
                # last attention pair
                WO = wop.tile([P, KC, H], fp8)
                nc.sync.dma_start(WO[:], woT_d[:])

                # ---- output projection + layernorm: emitted inside the
                # ---- attention PSUM scope (hp reuses the projection tag) so
                # ---- the scheduler can hoist matmuls into pair 7's
                # ---- ACT-bound stretch as PE filler.  qt 0-3 (query cols
                # ---- 0:512 = chunk c=0 of every pair) are emitted between
                # ---- pair 7's two halves so their out-proj/LN overlaps the
                # ---- c=1 attention instead of serializing after it.
                with tc.tile_pool(name="epi2", bufs=3) as epi, \
                     tc.tile_pool(name="tmps", bufs=8) as tmps_pool:
                    if use_affine:
                        GAM = epi.tile([P, H], f32, tag="gam")
                        BET = epi.tile([P, H], f32, tag="bet")
                        nc.sync.dma_start(GAM[:], gam_d[:])
                        nc.sync.dma_start(BET[:], bet_d[:])

                    tmps = {}

                    def epi_mm(qt):
                        # out-projection + residual add for one query tile;
                        # LN is deferred so these can interleave with pair
                        # 7's c=1 attention (no Sqrt near the exps -> no
                        # act-table thrash)
                        xqt = epi.tile([P, H], f32, tag="xq")
                        nc.sync.dma_start(xqt[:], xq_r[:, qt, :])
                        tmp = tmps_pool.tile([P, H], f32, tag="tmp",
                                             name=f"tmp_{qt}")
                        tmps[qt] = tmp
                        for jc in range(2):
                            hp = ppsum.tile([P, 512], f32, tag="pp",
                                            name=f"hps_{qt}_{jc}")
                            for k2 in range(KC // 2):
                                nc.tensor.matmul(
                                    hp[:],
                                    CTX[:, 2 * k2:2 * k2 + 2,
                                        qt * P:(qt + 1) * P],
                                    WO[:, 2 * k2:2 * k2 + 2,
                                       jc * 512:(jc + 1) * 512],
                                    start=(k2 == 0),
                                    stop=(KC % 2 == 0 and
                                          k2 == KC // 2 - 1),
                                    perf_mode=DR)
                            if KC % 2:
                                nc.tensor.matmul(
                                    hp[:],
                                    CTX[:, KC - 1, qt * P:(qt + 1) * P],
                                    WO[:, KC - 1, jc * 512:(jc + 1) * 512],
                                    start=False, stop=True)
                            nc.vector.scalar_tensor_tensor(
                                tmp[:, jc * 512:(jc + 1) * 512], hp[:],
                                float(OUT_SCALE),
                                xqt[:, jc * 512:(jc + 1) * 512],
                                OP.mult, OP.add)

                    def epi_ln(qt):
                        tmp = tmps[qt]
                        stats = epi.tile([P, 2, 6], f32, tag="st")
                        mv = epi.tile([P, 2], f32, tag="mv")
                        for c in range(2):
                            nc.vector.bn_stats(
                                stats[:, c, :],
                                tmp[:, c * 512:(c + 1) * 512])
                        nc.vector.bn_aggr(mv[:], stats[:])
                        ve = epi.tile([P, 1], f32, tag="ve")
                        nc.vector.tensor_scalar_add(ve[:], mv[:, 1:2],
                                                    float(EPS))
                        sd = epi.tile([P, 1], f32, tag="sd")
                        nc.scalar.activation(sd[:], ve[:], AF.Sqrt)
                        rstd = epi.tile([P, 1], f32, tag="rstd")
                        nc.vector.reciprocal(rstd[:], sd[:])
                        osb = epi.tile([P, H], f32, tag="osb")
                        nc.vector.tensor_scalar(
                            osb[:], tmp[:], mv[:, 0:1], rstd[:],
                            OP.subtract, OP.mult)
                        if use_affine:
                            nc.vector.tensor_tensor(osb[:], osb[:],
                                                    GAM[:], OP.mult)
                            nc.vector.tensor_tensor(osb[:], osb[:],
                                                    BET[:], OP.add)
                        nc.sync.dma_start(out_r[:, qt, :], osb[:])

                    emit_pair(7, crange=(0,))
                    # qt 0-3 (query cols 0:512, complete once every pair's
                    # c=0 has drained) fill pair 7's c=1 attention; the
                    # leading no-ops delay them past c=0's CTX drains so
                    # they never block c=1 score matmuls
                    qt_fill = [lambda: None] * 8 + \
                        [lambda qt=qt: epi_mm(qt) for qt in range(4)]
                    emit_pair(7, qt_fill, crange=(1,))
                    for qt in range(4):
                        epi_ln(qt)
                    for qt in range(4, 8):
                        epi_mm(qt)
                        epi_ln(qt)

    nc.compile()
    return nc


def _get_program(use_bias, use_affine):
    key = (use_bias, use_affine)
    if key not in _CACHE:
        _CACHE[key] = _build_program(use_bias, use_affine)
    return _CACHE[key]


def _prep_inputs(input_tensor, Wq, bq, Wk, bk, Wv, bv, Wo, bo, gamma, beta,
                 use_bias, use_affine):
    f8 = ml_dtypes.float8_e4m3
    x = np.asarray(input_tensor, np.float32)
    HP = H + P if use_bias else H

    KO = HP // P
    KC = H // P + (1 if use_bias else 0)

    def padw(w, b):
        m = np.zeros((HP, H), np.float32)
        m[:H] = np.asarray(w, np.float32).T * W_SCALE
        if use_bias:
            m[H] = np.asarray(b, np.float32) * W_SCALE
        return m.astype(f8)

    def swz(m, nj, jw):
        # [KO*128, nj*jw] -> [128, nj, KO, jw]: one contiguous DMA
        # descriptor per partition per (nj) load slice
        return np.ascontiguousarray(
            m.reshape(KO, P, nj, jw).transpose(1, 2, 0, 3))

    wqT = swz(padw(Wq, bq), 8, P)
    wkT = swz(padw(Wk, bk), 8, P)
    wvT = swz(padw(Wv, bv), 2, 512)
    woT = np.ascontiguousarray(
        padw(Wo, bo)[:KC * P].reshape(KC, P, H).transpose(1, 0, 2))

    in_maps = []
    for core in range(NCORES):
        b, qh = core // 2, core % 2
        xb = x[b]
        # only the core's own query-half rows: K/V for these keys are
        # computed locally and the peer half arrives via on-device AllGather
        xT = np.zeros((HP, SQ), np.float32)
        xT[:H] = xb[qh * SQ:(qh + 1) * SQ].T
        if use_bias:
            xT[H] = 1.0
        m = {
            "xT": xT.astype(f8),
            "xq": np.ascontiguousarray(xb[qh * SQ:(qh + 1) * SQ]),
            "wqT": wqT, "wkT": wkT, "wvT": wvT, "woT": woT,
        }
        if use_affine:
            m["gam"] = np.ascontiguousarray(np.broadcast_to(
                np.asarray(gamma, np.float32), (P, H)))
            m["bet"] = np.ascontiguousarray(np.broadcast_to(
                np.asarray(beta, np.float32), (P, H)))
        in_maps.append(m)
    return in_maps


def run(inputs, trace=False, tmpdir=None):
    from concourse.bass_utils import run_bass_kernel_spmd
    use_bias = any(
        np.any(np.asarray(inputs[k], np.float32) != 0.0)
        for k in ("bq", "bk", "bv", "bo"))
    use_affine = bool(
        np.any(np.asarray(inputs["gamma"], np.float32) != 1.0)
        or np.any(np.asarray(inputs["beta"], np.float32) != 0.0))
    nc = _get_program(use_bias, use_affine)
    in_maps = _prep_inputs(use_bias=use_bias, use_affine=use_affine, **inputs)
    res = run_bass_kernel_spmd(nc, in_maps, list(range(NCORES)), trace=trace,
                               tmpdir=tmpdir)
    out = np.zeros((B, S, H), np.float32)
    for core in range(NCORES):
        b, qh = core // 2, core % 2
        out[b, qh * SQ:(qh + 1) * SQ] = res.results[core]["out"]
    return out, res


def kernel(**inputs):
    out, _ = run(inputs, trace=False)
    return out



# revision 42
# speedup vs baseline: 1.1961x; 1.1961x over previous
"""Trainium2 Bass kernel for a full attention block (B=4, S=2048, H=1024, 16 heads).

Sharding: 8 cores = (batch b = core//2) x (query-half qh = core%2).
Each core computes the complete pipeline for its 1024 query rows of batch b:
QKV projections, 16-head attention over all 2048 keys, output projection,
residual add and layernorm.  No cross-core communication; the host slices
inputs and concatenates the 8 disjoint output shards.

v3 (stall-shaped PE stream), tuned from v2's trace (407us) where the PE
pipe was 100% occupied but ~20% of it was idle gaps + cold-clock matmuls
caused by in-order head-of-line stalls:
  - All four projections and the probs@V / ctx@Wo matmuls run as fp8e4
    DoubleRow matmuls (two 128-deep k-tiles per instruction).  Weights are
    pre-scaled x4 so their sigma ~ 0.124 sits in fp8e4's normal range; the
    compensation folds into the softmax exp scale and the residual add.
    Scores (contraction = head dim 64, un-pairable) stay bf16 and row-tile
    the two heads of a pair into array rows 0-63 / 64-127 (concurrent).
  - Softmax probs are fp8e5; exps split across ACT (Exp activation) and
    DVE (Schraudolph bitcast exp) by DVE_KTS; V PSUM drains ride the ACT
    engine (Copy lives in every act table set) to balance the two.
  - The PE stream is shaped around its in-order execution: per pair, ctx
    matmuls trail the score stream by LEAD k-tiles (late enough that the
    previous chunk's PSUM drain finished - no head-of-line stall - early
    enough to fill the exp-rate-limited score gaps), and Q/K/V projection
    tiles for FUTURE pairs are sprinkled between score k-tiles as filler.
  - ctx accumulates unnormalized with a ones column at V slot 0, so the
    softmax denominator L lands in PSUM partition 0 where the Pool
    partition_broadcast can read the DVE reciprocal directly (no
    cross-partition DMA bounce on the critical path); the 1/L multiply is
    fused into the PSUM->SBUF fp8 drain whose CTX DMA is off-path.
  - The epilogue (out-proj + residual + LN) is split: qt 0-3's matmul +
    residual interleave with pair 7's c=1 attention; layernorm (whose Sqrt
    would thrash the ACT exp table) runs after the last exp.
  - Weights arrive host-pre-swizzled so each SBUF weight load is one
    contiguous DMA descriptor per partition.
"""

import numpy as np
import ml_dtypes

B, S, H, NH, DH = 4, 2048, 1024, 16, 64
P = 128
NCORES = 8
SQ = 1024        # query rows per core
EPS = 1e-12

W_SCALE = 4.0                       # weight pre-scale into fp8e4 range
EXP_SCALE = 0.125 / (W_SCALE * W_SCALE)   # 1/sqrt(DH) / (Wq*Wk scales)
OUT_SCALE = 1.0 / (W_SCALE * W_SCALE)     # undo Wv*Wo scales after out-proj
LOG2E = 1.4426950408889634
# Schraudolph exp -> e5m2: i8 = sp * EXP_A + EXP_B, bitcast to fp8e5.
# value = 2^((i-60)/4); a = 4*EXP_SCALE*log2(e); b centers the sawtooth
# (-0.17) and compensates truncation (+0.5 if the convert truncates).
EXP_A = 4.0 * EXP_SCALE * LOG2E
EXP_B = 60.0 - 0.172
DVE_KTS = (2, 5, 8, 11, 14)         # score k-tiles whose exp runs on DVE

_CACHE = {}


def _build_program(use_bias, use_affine):
    from concourse import bacc, tile, mybir

    f32 = mybir.dt.float32
    bf16 = mybir.dt.bfloat16
    fp8 = mybir.dt.float8e4
    fp8e5 = mybir.dt.float8e5
    i8 = mybir.dt.int8
    AF = mybir.ActivationFunctionType
    OP = mybir.AluOpType
    DR = mybir.MatmulPerfMode.DoubleRow

    HP = H + P if use_bias else H   # padded contraction (bias ones row)
    KO = HP // P                    # projection contraction subtiles
    KC = H // P + (1 if use_bias else 0)  # out-proj contraction subtiles

    nc = bacc.Bacc("TRN2", target_bir_lowering=False, debug=False,
                   num_devices=NCORES)

    # weights arrive pre-swizzled from the host so every SBUF weight load is
    # one contiguous descriptor per partition (instead of a KO-fragment
    # descriptor storm that serializes the DMA queues at startup)
    xT_d = nc.dram_tensor("xT", [HP, S], fp8, kind="ExternalInput").ap()
    xq_d = nc.dram_tensor("xq", [SQ, H], f32, kind="ExternalInput").ap()
    wqT_d = nc.dram_tensor("wqT", [P, 8, KO, P], fp8,
                           kind="ExternalInput").ap()
    wkT_d = nc.dram_tensor("wkT", [P, 8, KO, P], fp8,
                           kind="ExternalInput").ap()
    wvT_d = nc.dram_tensor("wvT", [P, 2, KO, 512], fp8,
                           kind="ExternalInput").ap()
    woT_d = nc.dram_tensor("woT", [P, KC, H], fp8, kind="ExternalInput").ap()
    if use_affine:
        gam_d = nc.dram_tensor("gam", [P, H], f32, kind="ExternalInput").ap()
        bet_d = nc.dram_tensor("bet", [P, H], f32, kind="ExternalInput").ap()
    out_d = nc.dram_tensor("out", [SQ, H], f32, kind="ExternalOutput").ap()

    xT_r = xT_d.rearrange("(o p) s -> p o s", p=P)      # [128, KO, 2048]
    xq_r = xq_d.rearrange("(t p) j -> p t j", p=P)      # [128, 8, 1024]
    out_r = out_d.rearrange("(t p) j -> p t j", p=P)

    with tile.TileContext(nc) as tc:
        with tc.tile_pool(name="pers", bufs=1) as pers, \
             tc.tile_pool(name="probs", bufs=20) as probs_pool, \
             tc.tile_pool(name="att1", bufs=2) as att1, \
             tc.tile_pool(name="wop", bufs=1) as wop:
            KT = pers.tile([P, 8, S], bf16)       # [p, jt, s]; j = jt*128+p
            QT = pers.tile([P, 8, SQ], bf16)
            V = pers.tile([P, 16, NH, 66], fp8)   # [k%128, k//128, head, 1@0|d]
            CTX = pers.tile([P, KC, SQ], fp8)     # ctxT (+ ones row subtile)

            # ones column at V slot 0 -> softmax denominator L lands in PSUM
            # partition 0, where partition_broadcast can read it directly
            # (no cross-partition DMA bounce on the critical path)
            nc.gpsimd.memset(V[:, :, :, 0:1], 1.0)
            if use_bias:
                nc.gpsimd.memset(CTX[:, KC - 1, :], 0.0)
                # ones value W_SCALE so bias row (x4) reaches 16*bo
                nc.gpsimd.memset(CTX[0:1, KC - 1, :], W_SCALE)

            with tc.tile_pool(name="spsum", bufs=2, space="PSUM") as spsum, \
                 tc.tile_pool(name="ppsum", bufs=2, space="PSUM") as ppsum, \
                 tc.tile_pool(name="cpsum", bufs=2, space="PSUM") as cpsum:

                def emit_pair(pair, fillers=(), crange=(0, 1)):
                    hA, hB = 2 * pair, 2 * pair + 1
                    jt = pair
                    fillers = list(fillers)
                    nfill = len(fillers)
                    nslots = 16 * len(crange)
                    # two q-chunk halves; each [128,1024] score psum tile
                    # holds both heads (cols 0:512 head A, 512: head B) so a
                    # single exp op serves the pair; probs tiles pack kt
                    # PAIRS ([p, 2, 1024]) for the DoubleRow ctx matmuls.
                    #
                    # All score matmuls for both chunks are emitted BEFORE
                    # any ctx matmul: the PE executes its stream in order,
                    # so a ctx matmul waiting on a PSUM bank (previous
                    # chunk's drain) must not sit in front of independent
                    # score work (head-of-line blocking).  `fillers` are
                    # projection-tile thunks for upcoming pairs, sprinkled
                    # between score k-tiles: the score stream is rate-limited
                    # by the exp engines via the two sp PSUM banks, and the
                    # filler matmuls keep the PE busy (and its pstate hot)
                    # through each exp wait.
                    prs = {}
                    slot = 0
                    done = 0
                    LEAD = 6   # ctx matmuls trail the score stream by 6
                    #            k-tiles: late enough that the previous
                    #            chunk's PSUM drain is finished (no
                    #            head-of-line stall), early enough to fill
                    #            the exp-rate-limited score gaps

                    def ctx_mm(c, m):
                        t2, hi = m // 2, m % 2
                        h = (hA, hB)[hi]
                        base = hi * 64
                        nc.tensor.matmul(
                            ctxpss[c][h][:],
                            V[:, 2 * t2:2 * t2 + 2, h, 0:65],
                            prs[c, t2][:, 0:2, base * 8:base * 8 + 512],
                            start=(t2 == 0), stop=(t2 == 7),
                            perf_mode=DR)

                    ctxpss = {}
                    for c in crange:
                        cs = slice(c * 512, (c + 1) * 512)
                        ctxpss[c] = {h: cpsum.tile([65, 512], f32, tag="ctx",
                                                   name=f"ctxps_{h}_{c}")
                                     for h in (hA, hB)}
                        for kt in range(16):
                            t2, j2 = kt // 2, kt % 2
                            if j2 == 0:
                                prs[c, t2] = probs_pool.tile(
                                    [P, 2, SQ], fp8e5, tag="pt",
                                    name=f"pr_{t2}_{c}")
                            pr = prs[c, t2]
                            sp = spsum.tile([P, SQ], f32, tag="sp",
                                            name=f"sps_{kt}_{c}")
                            for h in (hA, hB):
                                base = (h % 2) * 64
                                nc.tensor.matmul(
                                    sp[:, base * 8:base * 8 + 512],
                                    KT[base:base + 64, jt, kt * P:(kt + 1) * P],
                                    QT[base:base + 64, jt, cs],
                                    start=True, stop=True)
                            if kt in DVE_KTS:
                                nc.vector.tensor_scalar(
                                    pr[:, j2, :].bitcast(i8), sp[:],
                                    float(EXP_A), float(EXP_B),
                                    OP.mult, OP.add)
                            else:
                                nc.scalar.activation(pr[:, j2, :], sp[:],
                                                     AF.Exp,
                                                     scale=float(EXP_SCALE))
                            if kt >= LEAD:
                                ctx_mm(c, kt - LEAD)
                            slot += 1
                            want = (nfill * slot) // nslots
                            while done < want:
                                fillers[done]()
                                done += 1
                        for m in range(16 - LEAD, 16):
                            ctx_mm(c, m)
                    while done < nfill:
                        fillers[done]()
                        done += 1
                    for c in crange:
                        cs = slice(c * 512, (c + 1) * 512)
                        ctxps = ctxpss[c]
                        # denominator L = psum row 0 (V ones column at slot
                        # 0): copy to SBUF partition 0 (the custom recip op
                        # misreads PSUM on hw), 1/L, Pool-broadcast straight
                        # from partition 0, and fold the multiply into the
                        # PSUM->SBUF fp8 drain; both ctx halves bounce
                        # through an SBUF stage whose DMA to CTX is off the
                        # critical path.
                        for h in (hA, hB):
                            base = (h % 2) * 64
                            lstage = att1.tile([1, 512], f32, tag="lstage",
                                               name=f"lstage_{h}_{c}")
                            nc.vector.tensor_copy(lstage[:],
                                                  ctxps[h][0:1, :])
                            lrec = att1.tile([1, 512], f32, tag="lrec",
                                             name=f"lrec_{h}_{c}")
                            nc.vector.reciprocal_approx_fast(lrec[:],
                                                             lstage[:])
                            lrep = att1.tile([80, 512], f32, tag="lrep",
                                             name=f"lrep_{h}_{c}")
                            nc.gpsimd.partition_broadcast(lrep[:],
                                                          lrec[0:1, :])
                            # stt over rows 0:65 (row 0 computes L*(1/L),
                            # discarded) -- PSUM reads must start at an
                            # aligned partition, so cover the full range and
                            # DMA only rows 1:65 into CTX
                            cstage = att1.tile([65, 512], fp8,
                                               tag="cstage",
                                               name=f"cstage_{h}_{c}")
                            nc.vector.scalar_tensor_tensor(
                                cstage[0:65, :], ctxps[h][0:65, :],
                                1.0, lrep[0:65, :], OP.bypass, OP.mult)
                            nc.sync.dma_start(CTX[base:base + 64, jt, cs],
                                              cstage[1:65, :])

                with tc.tile_pool(name="proj", bufs=1) as projp, \
                     tc.tile_pool(name="wstr", bufs=2) as wstr, \
                     tc.tile_pool(name="wvstr", bufs=1) as wvstr:
                    XT = projp.tile([P, KO, S], fp8)
                    # issue the first Q-weight tile ahead of the bulk XT
                    # chunks so the PE can start at chunk 1, not chunk 8
                    wq0_t = wstr.tile([P, KO, P], fp8, tag="w", name="wq_0")
                    nc.sync.dma_start(wq0_t[:], wqT_d[:, 0, :, :])
                    # per-subtile loads so the first projection matmul can
                    # start as soon as chunk 0 lands; chunk 0 is split into
                    # 4 partition-range DMAs so it arrives ~4x sooner
                    for ps in range(0, P, 32):
                        nc.sync.dma_start(XT[ps:ps + 32, 0, :],
                                          xT_r[ps:ps + 32, 0, :])
                    for ko in range(1, KO):
                        nc.sync.dma_start(XT[:, ko, :], xT_r[:, ko, :])

                    def proj_mm(ps, w_t, cs):
                        for k2 in range(KO // 2):
                            nc.tensor.matmul(
                                ps[:], w_t[:, 2 * k2:2 * k2 + 2, :],
                                XT[:, 2 * k2:2 * k2 + 2, cs],
                                start=(k2 == 0),
                                stop=(KO % 2 == 0 and k2 == KO // 2 - 1),
                                perf_mode=DR)
                        if KO % 2:
                            nc.tensor.matmul(
                                ps[:], w_t[:, KO - 1, :], XT[:, KO - 1, cs],
                                start=False, stop=True)

                    def qk_thunks(jt, wq_t=None):
                        # weight DMAs issue now (well ahead); the per-tile
                        # matmul+drain emissions are returned as thunks for
                        # interleaving into the previous pair's score loop
                        if wq_t is None:
                            wq_t = wstr.tile([P, KO, P], fp8, tag="w",
                                             name=f"wq_{jt}")
                            nc.sync.dma_start(wq_t[:],
                                              wqT_d[:, jt, :, :])
                        wk_t = wstr.tile([P, KO, P], fp8, tag="w",
                                         name=f"wk_{jt}")
                        nc.sync.dma_start(wk_t[:],
                                          wkT_d[:, jt, :, :])

                        def q_tile(sc):
                            ps = ppsum.tile([P, 512], f32, tag="pp",
                                            name=f"qps_{jt}_{sc}")
                            proj_mm(ps, wq_t, slice(sc * 512, sc * 512 + 512))
                            nc.vector.tensor_copy(
                                QT[:, jt, sc * 512:(sc + 1) * 512], ps[:])

                        def k_tile(sc):
                            ps = ppsum.tile([P, 512], f32, tag="pp",
                                            name=f"kps_{jt}_{sc}")
                            proj_mm(ps, wk_t, slice(sc * 512, sc * 512 + 512))
                            nc.vector.tensor_copy(
                                KT[:, jt, sc * 512:(sc + 1) * 512], ps[:])

                        return ([lambda sc=sc: q_tile(sc) for sc in range(2)]
                                + [lambda sc=sc: k_tile(sc)
                                   for sc in range(4)])

                    def v_thunks(jc):
                        wv_t = wvstr.tile([P, KO, 512], fp8, tag="wv",
                                          name=f"wv_{jc}")
                        nc.sync.dma_start(
                            wv_t[:], wvT_d[:, jc, :, :])

                        def v_tile(st):
                            ps = ppsum.tile([P, 512], f32, tag="pp",
                                            name=f"vps_{st}_{jc}")
                            for k2 in range(KO // 2):
                                nc.tensor.matmul(
                                    ps[:],
                                    XT[:, 2 * k2:2 * k2 + 2, st * P:(st + 1) * P],
                                    wv_t[:, 2 * k2:2 * k2 + 2, :],
                                    start=(k2 == 0),
                                    stop=(KO % 2 == 0 and k2 == KO // 2 - 1),
                                    perf_mode=DR)
                            if KO % 2:
                                nc.tensor.matmul(
                                    ps[:], XT[:, KO - 1, st * P:(st + 1) * P],
                                    wv_t[:, KO - 1, :],
                                    start=False, stop=True)
                            # V drain on ACT (Copy is resident in every act
                            # table set) -- DVE is the more loaded engine
                            nc.scalar.activation(
                                V[:, st, jc * 8:(jc + 1) * 8, 1:65],
                                ps[:].rearrange("p (h d) -> p h d", d=64),
                                AF.Copy)

                        return [lambda st=st: v_tile(st) for st in range(16)]

                    # pair 0 needs QK(0) + V half 0 immediately; everything
                    # later streams in as filler.  V's second half feeds only
                    # pairs 4-7: its tiles fill pairs 2-3.
                    for th in qk_thunks(0, wq_t=wq0_t):
                        th()
                    for th in v_thunks(0):
                        th()
                    emit_pair(0, qk_thunks(1))
                    emit_pair(1, qk_thunks(2))
                    v1 = v_thunks(1)
                    emit_pair(2, qk_thunks(3) + v1[:10])
                    emit_pair(3, v1[10:] + qk_thunks(4))
                    emit_pair(4, qk_thunks(5))
                    emit_pair(5, qk_thunks(6))
                    emit_pair(6, qk_thunks(7))

                # XT freed; stream the output-projection weight during the
                # last attention pair
                WO = wop.tile([P, KC, H], fp8)
                nc.sync.dma_start(WO[:], woT_d[:])

                # ---- output projection + layernorm: emitted inside the
                # ---- attention PSUM scope (hp reuses the projection tag) so
                # ---- the scheduler can hoist matmuls into pair 7's
                # ---- ACT-bound stretch as PE filler.  qt 0-3 (query cols
                # ---- 0:512 = chunk c=0 of every pair) are emitted between
                # ---- pair 7's two halves so their out-proj/LN overlaps the
                # ---- c=1 attention instead of serializing after it.
                with tc.tile_pool(name="epi2", bufs=3) as epi, \
                     tc.tile_pool(name="tmps", bufs=8) as tmps_pool:
                    if use_affine:
                        GAM = epi.tile([P, H], f32, tag="gam")
                        BET = epi.tile([P, H], f32, tag="bet")
                        nc.sync.dma_start(GAM[:], gam_d[:])
                        nc.sync.dma_start(BET[:], bet_d[:])

                    tmps = {}

                    def epi_mm(qt):
                        # out-projection + residual add for one query tile;
                        # LN is deferred so these can interleave with pair
                        # 7's c=1 attention (no Sqrt near the exps -> no
                        # act-table thrash)
                        xqt = epi.tile([P, H], f32, tag="xq")
                        nc.sync.dma_start(xqt[:], xq_r[:, qt, :])
                        tmp = tmps_pool.tile([P, H], f32, tag="tmp",
                                             name=f"tmp_{qt}")
                        tmps[qt] = tmp
                        for jc in range(2):
                            hp = ppsum.tile([P, 512], f32, tag="pp",
                                            name=f"hps_{qt}_{jc}")
                            for k2 in range(KC // 2):
                                nc.tensor.matmul(
                                    hp[:],
                                    CTX[:, 2 * k2:2 * k2 + 2,
                                        qt * P:(qt + 1) * P],
                                    WO[:, 2 * k2:2 * k2 + 2,
                                       jc * 512:(jc + 1) * 512],
                                    start=(k2 == 0),
                                    stop=(KC % 2 == 0 and
                                          k2 == KC // 2 - 1),
                                    perf_mode=DR)
                            if KC % 2:
                                nc.tensor.matmul(
                                    hp[:],
                                    CTX[:, KC - 1, qt * P:(qt + 1) * P],
                                    WO[:, KC - 1, jc * 512:(jc + 1) * 512],
                                    start=False, stop=True)
                            nc.vector.scalar_tensor_tensor(
                                tmp[:, jc * 512:(jc + 1) * 512], hp[:],
                                float(OUT_SCALE),
                                xqt[:, jc * 512:(jc + 1) * 512],
                                OP.mult, OP.add)

                    def epi_ln(qt):
                        tmp = tmps[qt]
                        stats = epi.tile([P, 2, 6], f32, tag="st")
                        mv = epi.tile([P, 2], f32, tag="mv")
                        for c in range(2):
                            nc.vector.bn_stats(
                                stats[:, c, :],
                                tmp[:, c * 512:(c + 1) * 512])
                        nc.vector.bn_aggr(mv[:], stats[:])
                        ve = epi.tile([P, 1], f32, tag="ve")
                        nc.vector.tensor_scalar_add(ve[:], mv[:, 1:2],
                                                    float(EPS))
                        sd = epi.tile([P, 1], f32, tag="sd")
                        nc.scalar.activation(sd[:], ve[:], AF.Sqrt)
                        rstd = epi.tile([P, 1], f32, tag="rstd")
                        nc.vector.reciprocal(rstd[:], sd[:])
                        osb = epi.tile([P, H], f32, tag="osb")
                        nc.vector.tensor_scalar(
                            osb[:], tmp[:], mv[:, 0:1], rstd[:],
                            OP.subtract, OP.mult)
                        if use_affine:
                            nc.vector.tensor_tensor(osb[:], osb[:],
                                                    GAM[:], OP.mult)
                            nc.vector.tensor_tensor(osb[:], osb[:],
                                                    BET[:], OP.add)
                        nc.sync.dma_start(out_r[:, qt, :], osb[:])

                    emit_pair(7, crange=(0,))
                    # qt 0-3 (query cols 0:512, complete once every pair's
                    # c=0 has drained) fill pair 7's c=1 attention; the
                    # leading no-ops delay them past c=0's CTX drains so
                    # they never block c=1 score matmuls
                    qt_fill = [lambda: None] * 8 + \
                        [lambda qt=qt: epi_mm(qt) for qt in range(4)]
                    emit_pair(7, qt_fill, crange=(1,))
                    for qt in range(4):
                        epi_ln(qt)
                    for qt in range(4, 8):
                        epi_mm(qt)
                        epi_ln(qt)

    nc.compile()
    return nc


def _get_program(use_bias, use_affine):
    key = (use_bias, use_affine)
    if key not in _CACHE:
        _CACHE[key] = _build_program(use_bias, use_affine)
    return _CACHE[key]


def _prep_inputs(input_tensor, Wq, bq, Wk, bk, Wv, bv, Wo, bo, gamma, beta,
                 use_bias, use_affine):
    f8 = ml_dtypes.float8_e4m3
    x = np.asarray(input_tensor, np.float32)
    HP = H + P if use_bias else H

    KO = HP // P
    KC = H // P + (1 if use_bias else 0)

    def padw(w, b):
        m = np.zeros((HP, H), np.float32)
        m[:H] = np.asarray(w, np.float32).T * W_SCALE
        if use_bias:
            m[H] = np.asarray(b, np.float32) * W_SCALE
        return m.astype(f8)

    def swz(m, nj, jw):
        # [KO*128, nj*jw] -> [128, nj, KO, jw]: one contiguous DMA
        # descriptor per partition per (nj) load slice
        return np.ascontiguousarray(
            m.reshape(KO, P, nj, jw).transpose(1, 2, 0, 3))

    wqT = swz(padw(Wq, bq), 8, P)
    wkT = swz(padw(Wk, bk), 8, P)
    wvT = swz(padw(Wv, bv), 2, 512)
    woT = np.ascontiguousarray(
        padw(Wo, bo)[:KC * P].reshape(KC, P, H).transpose(1, 0, 2))

    in_maps = []
    for core in range(NCORES):
        b, qh = core // 2, core % 2
        xb = x[b]
        rolled = np.concatenate(
            [xb[qh * SQ:(qh + 1) * SQ], xb[(1 - qh) * SQ:(2 - qh) * SQ]], 0)
        xT = np.zeros((HP, S), np.float32)
        xT[:H] = rolled.T
        if use_bias:
            xT[H] = 1.0
        m = {
            "xT": xT.astype(f8),
            "xq": np.ascontiguousarray(xb[qh * SQ:(qh + 1) * SQ]),
            "wqT": wqT, "wkT": wkT, "wvT": wvT, "woT": woT,
        }
        if use_affine:
            m["gam"] = np.ascontiguousarray(np.broadcast_to(
                np.asarray(gamma, np.float32), (P, H)))
            m["bet"] = np.ascontiguousarray(np.broadcast_to(
                np.asarray(beta, np.float32), (P, H)))
        in_maps.append(m)
    return in_maps


def run(inputs, trace=False, tmpdir=None):
    from concourse.bass_utils import run_bass_kernel_spmd
    use_bias = any(
        np.any(np.asarray(inputs[k], np.float32) != 0.0)
        for k in ("bq", "bk", "bv", "bo"))
    use_affine = bool(
        np.any(np.asarray(inputs["gamma"], np.float32) != 1.0)
        or np.any(np.asarray(inputs["beta"], np.float32) != 0.0))
    nc = _get_program(use_bias, use_affine)
    in_maps = _prep_inputs(use_bias=use_bias, use_affine=use_affine, **inputs)
    res = run_bass_kernel_spmd(nc, in_maps, list(range(NCORES)), trace=trace,
                               tmpdir=tmpdir)
    out = np.zeros((B, S, H), np.float32)
    for core in range(NCORES):
        b, qh = core // 2, core % 2
        out[b, qh * SQ:(qh + 1) * SQ] = res.results[core]["out"]
    return out, res


def kernel(**inputs):
    out, _ = run(inputs, trace=False)
    return out

